# revision 32
# baseline (speedup 1.0000x reference)
"""CGC MoE routing kernel for Trainium2, 8-core data-parallel over batch.

Problem (per reference):
  B=4096, D_FULL=1024, D_T1=D_T2=512, experts: 4 shared (on x_full),
  4 task-1 (on x_task1), 4 task-2 (on x_task2); each expert is a 2-layer
  ReLU MLP (hidden 512, out 256). Three softmax gates combine expert
  outputs into (out_sh, out1, out2), each [4096, 256] fp32.

Strategy: shard the batch 8 ways (512 rows/core), replicate weights.
Each core computes all 12 experts + gates for its shard; host concats.
Matmuls run in MM_DT (bfloat16 by default, ~3e-3 rel err; float32r gives
~2e-4 at ~20% more time) with fp32 PSUM accumulation.  Matmul operands
are cast from fp32 during their SWDGE DMA; DMAs are heavily batched
because each SWDGE issue costs ~1us on the GpSimd engine.

Layout: activations kept as [feature, batch]; the host pre-transposes
the x shards and pre-packs weights into SBUF partition layout so every
DMA is a long contiguous run per partition.
  L1: H[h,b]  = sum_d W1[d,h].T @ X'[d,b]      (W1 tiles stationary)
  L2: EO[b,o] = sum_h H[h,b].T  @ W2[h,o]      (H tiles stationary)
      + ones[1,b].T @ b2[1,o] K=1 matmul for the bias, then ReLU.
  Gates: logits[g,b] = sum_d gW[d,g].T @ X'[d,b]; exp with per-partition
      bias on ACT; PE-transpose to [b,g]; columns pre-scaled by 1/rowsum
      so the combine uses normalized gates directly.
  Combine: acc[b,o] (+)= EO_e[b,o] * gate_col[b,1] on DVE
      (scalar_tensor_tensor fused multiply-add); acc DMAs straight out.
"""
import os as _os
import sys
import numpy as np

sys.path.insert(0, "/opt/trn_rl_repo")

import concourse.bass as bass
import concourse.mybir as mybir
import concourse.tile as tile
import concourse.masks as masks
from concourse.bass_utils import run_bass_kernel_spmd

F32 = mybir.dt.float32
MM_DT = (mybir.dt.float32r if _os.environ.get("MOE_MM_DT") == "fp32r"
         else mybir.dt.bfloat16)

B = 4096
N_CORES = 8
BC = B // N_CORES          # 512 rows per core
DF, D1, D2 = 1024, 512, 512
HID, OUT = 512, 256
NB = BC // 128             # 4 batch tiles per core
NH = HID // 128            # 4 hidden tiles
E = 4                      # experts per group
NDS = {"t1": D1 // 128, "sh": DF // 128, "t2": D2 // 128}
W1SZ = {g: nd * HID for g, nd in NDS.items()}
WSZ = {g: W1SZ[g] + NH * OUT for g in NDS}   # combined w1+w2 free size


def _legalize_waits(nc, max_waits: int = 1):
    """This walrus build supports a single sync wait per instruction;
    hoist extra waits onto standalone single-wait EventSemaphore
    instructions inserted just before (same engine, same order)."""
    uid = 0
    for f in nc.m.functions:
        for blk in f.blocks:
            out = []
            changed = False
            for inst in blk.instructions:
                si = inst.sync_info
                ow = list(si.on_wait) if si and si.on_wait else []
                if len(ow) > max_waits:
                    changed = True
                    for w in ow[:-max_waits]:
                        ev = mybir.InstEventSemaphore(
                            name=f"legalw-{uid}",
                            sync_info=mybir.SyncInfo(on_wait=[w], on_update=[]),
                        )
                        uid += 1
                        ev.engine = inst.engine
                        out.append(ev)
                    inst.sync_info = mybir.SyncInfo(
                        on_wait=ow[-max_waits:],
                        on_update=list(si.on_update) if si.on_update else [],
                    )
                out.append(inst)
            if changed:
                blk.instructions = out
    return nc


def _build_nc():
    nc = bass.Bass()

    def din(name, shape):
        return nc.declare_dram_parameter(name, list(shape), F32, isOutput=False)

    # x shards packed [128, (d_tile, b)]
    xfT = din("xfT", (128, NDS["sh"] * BC))
    x1T = din("x1T", (128, NDS["t1"] * BC))
    x2T = din("x2T", (128, NDS["t2"] * BC))
    # per-expert combined weights [E, 128, (nd*HID + NH*OUT)]
    t1W = din("t1W", (E, 128, WSZ["t1"]))
    shW = din("shW", (E, 128, WSZ["sh"]))
    t2W = din("t2W", (E, 128, WSZ["t2"]))
    # biases: b1 packed per group [128, E*NH]; b2 per group [E, OUT]
    t1B1 = din("t1B1", (128, E * NH)); t1B2 = din("t1B2", (1, E * OUT))
    shB1 = din("shB1", (128, E * NH)); shB2 = din("shB2", (1, E * OUT))
    t2B1 = din("t2B1", (128, E * NH)); t2B2 = din("t2B2", (1, E * OUT))
    # gate weights packed [128, nd*ng]; all gate biases packed [28, 1]
    gshW = din("gshW", (128, NDS["sh"] * 12))
    g1W = din("g1W", (128, NDS["t1"] * 8))
    g2W = din("g2W", (128, NDS["t2"] * 8))
    gB = din("gB", (96, 1))

    out_sh = nc.declare_dram_parameter("out_sh", [BC, OUT], F32, isOutput=True)
    out1 = nc.declare_dram_parameter("out1", [BC, OUT], F32, isOutput=True)
    out2 = nc.declare_dram_parameter("out2", [BC, OUT], F32, isOutput=True)

    with tile.TileContext(nc) as tc:
        _emit(nc, tc,
              {"xf": xfT, "x1": x1T, "x2": x2T},
              # expert groups in processing order: t1, sh, t2
              [("t1", t1W, t1B1, t1B2),
               ("sh", shW, shB1, shB2),
               ("t2", t2W, t2B1, t2B2)],
              [(gshW, 12), (g1W, 8), (g2W, 8)], gB,
              [out_sh, out1, out2])
    _legalize_waits(nc)
    return nc


def _emit(nc, tc, xins, expert_groups, gate_params, gB, outs):
    from contextlib import ExitStack
    ctx = ExitStack()
    with ctx:
        xp = ctx.enter_context(tc.tile_pool(name="xp", bufs=1))
        wp = ctx.enter_context(tc.tile_pool(name="wp", bufs=3))
        bp = ctx.enter_context(tc.tile_pool(name="bp", bufs=1))
        hp = ctx.enter_context(tc.tile_pool(name="hp", bufs=2))
        eop = ctx.enter_context(tc.tile_pool(name="eop", bufs=12))
        gp = ctx.enter_context(tc.tile_pool(name="gp", bufs=1))
        accp = ctx.enter_context(tc.tile_pool(name="accp", bufs=1))
        misc = ctx.enter_context(tc.tile_pool(name="misc", bufs=1))
        # PSUM: 8 banks; L1 and gate logits share tag p1, L2 and gate
        # transposes share tag p2.
        ps1 = ctx.enter_context(tc.tile_pool(name="ps1", bufs=4, space="PSUM"))
        ps2 = ctx.enter_context(tc.tile_pool(name="ps2", bufs=4, space="PSUM"))

        # ---- batched loads ---------------------------------------------
        def load_xT(key):
            t = xp.tile([128, NDS[{"x1": "t1", "xf": "sh", "x2": "t2"}[key]] * BC],
                        MM_DT, name=f"x_{key}", tag=f"x_{key}")
            nc.gpsimd.dma_start(t[:], xins[key][:])
            return t

        def load_w(W, group, e):
            w = wp.tile([128, WSZ[group]], MM_DT, name="w", tag="w")
            nc.gpsimd.dma_start(w[:], W[e])
            return w

        # x1 + first expert's weights lead the SWDGE queue
        x1 = load_xT("x1")
        xts = {"t1": x1}
        g0, W_0, B1_0, B2_0 = expert_groups[0]
        w_e0 = load_w(W_0, g0, 0)
        xts["sh"] = load_xT("xf")
        xts["t2"] = load_xT("x2")

        # biases (HWDGE, fp32) and b2 (SWDGE, MM_DT) per group, one DMA each
        b1t, b2t = {}, {}
        for group, W, B1, B2 in expert_groups:
            b1t[group] = bp.tile([128, E * NH], F32, name=f"b1_{group}",
                                 tag=f"b1_{group}")
            nc.sync.dma_start(b1t[group][:], B1[:])
            b2t[group] = bp.tile([1, E * OUT], MM_DT, name=f"b2_{group}",
                                 tag=f"b2_{group}")
            nc.gpsimd.dma_start(b2t[group][:], B2[:])
        gbt = bp.tile([96, 1], F32, name="gbt", tag="gbt")
        nc.sync.dma_start(gbt[:], gB[:])
        gb_sl = {0: gbt[0:12, :], 1: gbt[32:40, :], 2: gbt[64:72, :]}

        # identity for PE transpose; ones row for K=1 bias matmuls
        # (memset/iota are invalid ISA at fp32r/bf16 -> build fp32 + copy)
        ident32 = misc.tile([128, 128], F32, tag="ident32")
        masks.make_identity(nc, ident32[:])
        ident = misc.tile([128, 128], MM_DT, tag="ident")
        nc.vector.tensor_copy(ident[:], ident32[:])
        ones32 = misc.tile([1, 128], F32, tag="ones32")
        nc.vector.memset(ones32[:], 1.0)
        ones = misc.tile([1, 128], MM_DT, tag="ones")
        nc.vector.tensor_copy(ones[:], ones32[:])

        # accumulator tiles [128, OUT] per output per b-tile
        acc = [[accp.tile([128, OUT], F32, name=f"acc{o}_{bi}",
                          tag=f"acc{o}_{bi}")
                for bi in range(NB)] for o in range(3)]
        acc_init = [[False] * NB for _ in range(3)]

        # expert -> (output index, gate set, gate column) contributions
        # gates: gsh over [t1(0-3), t2(4-7), sh(8-11)]
        #        g1  over [t1(0-3), sh(4-7)]; g2 over [t2(0-3), sh(4-7)]
        def contributions(group, e):
            if group == "t1":
                return [(0, 0, e), (1, 1, e)]
            elif group == "t2":
                return [(0, 0, 4 + e), (2, 2, e)]
            else:
                return [(0, 0, 8 + e), (1, 1, 4 + e), (2, 2, 4 + e)]

        # processing order: t1(0..3), sh(0..3), t2(0..3)
        def _is_last_contrib(group, e, o):
            if o == 1:
                return group == "sh" and e == E - 1
            return group == "t2" and e == E - 1

        # ---- expert bodies ---------------------------------------------
        def emit_expert_l1(group, e, w):
            xt = xts[group]
            nd = NDS[group]
            b1 = b1t[group]
            h = hp.tile([128, NH * BC], MM_DT, name="h", tag="h")
            for hi in range(NH):
                p1 = ps1.tile([128, BC], F32, name="p1", tag="p1")
                for di in range(nd):
                    nc.tensor.matmul(
                        p1[:], w[:, di * HID + hi * 128: di * HID + (hi + 1) * 128],
                        xt[:, di * BC:(di + 1) * BC],
                        start=(di == 0), stop=(di == nd - 1))
                nc.scalar.activation(h[:, hi * BC:(hi + 1) * BC], p1[:],
                                     mybir.ActivationFunctionType.Relu,
                                     bias=b1[:, e * NH + hi: e * NH + hi + 1])
            return h

        def emit_expert_l2c(group, e, h, w):
            w2off = W1SZ[group]
            eos = []
            for bi in range(NB):
                p2 = ps2.tile([128, OUT], F32, name="p2", tag="p2")
                for hi in range(NH):
                    nc.tensor.matmul(
                        p2[:],
                        h[:, hi * BC + bi * 128: hi * BC + (bi + 1) * 128],
                        w[:, w2off + hi * OUT: w2off + (hi + 1) * OUT],
                        start=(hi == 0), stop=False)
                nc.tensor.matmul(p2[:], ones[:],
                                 b2t[group][:, e * OUT:(e + 1) * OUT],
                                 start=False, stop=True)
                eo = eop.tile([128, OUT], F32, name="eo", tag="eo")
                nc.scalar.activation(eo[:], p2[:],
                                     mybir.ActivationFunctionType.Relu)
                eos.append(eo)
            return eos

        def emit_combine(group, e, eos, gate_cols):
            for bi in range(NB):
                eo = eos[bi]
                for (o, gs, col) in contributions(group, e):
                    g_ap = gate_cols[gs][bi][:, col:col + 1]
                    a = acc[o][bi]
                    if not acc_init[o][bi]:
                        nc.vector.tensor_scalar_mul(a[:], eo[:], g_ap)
                        acc_init[o][bi] = True
                    else:
                        nc.vector.scalar_tensor_tensor(
                            a[:], eo[:], g_ap, a[:],
                            op0=mybir.AluOpType.mult,
                            op1=mybir.AluOpType.add)
                    if _is_last_contrib(group, e, o):
                        nc.sync.dma_start(
                            outs[o][bi * 128:(bi + 1) * 128, :], a[:])

        # ---- gates ------------------------------------------------------
        def emit_gates():
            gate_cols = []
            for gi, (gW, ng) in enumerate(gate_params):
                xt = {0: xts["sh"], 1: xts["t1"], 2: xts["t2"]}[gi]
                nd = {0: NDS["sh"], 1: NDS["t1"], 2: NDS["t2"]}[gi]
                gwt = gp.tile([128, nd * ng], MM_DT, name=f"gw{gi}",
                              tag=f"gw{gi}")
                nc.gpsimd.dma_start(gwt[:], gW[:])
                lg = ps1.tile([ng, BC], F32, name="lg", tag="p1")
                for di in range(nd):
                    nc.tensor.matmul(
                        lg[:], gwt[:, di * ng:(di + 1) * ng],
                        xt[:, di * BC:(di + 1) * BC],
                        start=(di == 0), stop=(di == nd - 1))
                eg = gp.tile([ng, BC], MM_DT, name=f"eg{gi}", tag=f"eg{gi}")
                nc.scalar.activation(eg[:], lg[:],
                                     mybir.ActivationFunctionType.Exp,
                                     bias=gb_sl[gi])
                cols = []
                for bi in range(NB):
                    pt = ps2.tile([128, ng], MM_DT, name="gtr", tag="p2")
                    nc.tensor.transpose(pt[:], eg[:, bi * 128:(bi + 1) * 128],
                                        ident[:ng, :ng])
                    ct = gp.tile([128, ng], F32, name=f"gc{gi}_{bi}",
                                 tag=f"gc{gi}_{bi}")
                    nc.vector.tensor_copy(ct[:], pt[:])
                    st = gp.tile([128, 1], F32, name=f"gs{gi}_{bi}",
                                 tag=f"gs{gi}_{bi}")
                    nc.vector.tensor_reduce(st[:], ct[:],
                                            axis=mybir.AxisListType.X,
                                            op=mybir.AluOpType.add)
                    rt = gp.tile([128, 1], F32, name=f"gr{gi}_{bi}",
                                 tag=f"gr{gi}_{bi}")
                    nc.vector.reciprocal(rt[:], st[:])
                    # pre-scale: combine then uses normalized gates directly
                    nc.vector.tensor_scalar_mul(ct[:], ct[:], rt[:])
                    cols.append(ct)
                gate_cols.append(cols)
            return gate_cols

        # ---- emission order --------------------------------------------
        # t1_0 L1/L2 first (PE starts as soon as x1+w arrive), then t1_1,
        # then gates (xf has landed by then), deferred combines, rest.
        h_0 = emit_expert_l1(g0, 0, w_e0)
        eos_0 = emit_expert_l2c(g0, 0, h_0, w_e0)
        w_e1 = load_w(W_0, g0, 1)
        h_1 = emit_expert_l1(g0, 1, w_e1)
        eos_1 = emit_expert_l2c(g0, 1, h_1, w_e1)
        gate_cols = emit_gates()
        emit_combine(g0, 0, eos_0, gate_cols)
        emit_combine(g0, 1, eos_1, gate_cols)
        for gidx, (group, W, B1, B2) in enumerate(expert_groups):
            for e in range(E):
                if gidx == 0 and e in (0, 1):
                    continue
                w = load_w(W, group, e)
                h = emit_expert_l1(group, e, w)
                eos = emit_expert_l2c(group, e, h, w)
                emit_combine(group, e, eos, gate_cols)


_NC_CACHE = None


def _pack_inputs(inputs):
    """Host-side packing into SBUF partition layouts (pure relayout)."""
    f32 = lambda a: np.ascontiguousarray(a, dtype=np.float32)

    def pack_w(w1, w2):   # [E,D,HID],[E,HID,OUT] -> [E,128,nd*HID+NH*OUT]
        e, dd, _ = w1.shape
        nd = dd // 128
        a = w1.reshape(e, nd, 128, HID).transpose(0, 2, 1, 3).reshape(e, 128, nd * HID)
        b = w2.reshape(e, NH, 128, OUT).transpose(0, 2, 1, 3).reshape(e, 128, NH * OUT)
        return f32(np.concatenate([a, b], axis=2))

    def pack_b1(b):       # [E, HID] -> [128, E*NH] with [p, e*NH+n] = b[e, n*128+p]
        e, hh = b.shape
        nh = hh // 128
        return f32(b.reshape(e, nh, 128).transpose(2, 0, 1).reshape(128, e * nh))

    def pack_gw(w):       # [D, ng] -> [128, nd*ng]
        dd, ng = w.shape
        nd = dd // 128
        return f32(w.reshape(nd, 128, ng).transpose(1, 0, 2).reshape(128, nd * ng))

    gb = np.zeros(96, dtype=np.float32)
    gb[0:12] = inputs["gsh_b"]; gb[32:40] = inputs["g1_b"]; gb[64:72] = inputs["g2_b"]
    return {
        "t1W": pack_w(inputs["t1_W1"], inputs["t1_W2"]),
        "shW": pack_w(inputs["sh_W1"], inputs["sh_W2"]),
        "t2W": pack_w(inputs["t2_W1"], inputs["t2_W2"]),
        "t1B1": pack_b1(inputs["t1_b1"]), "t1B2": f32(inputs["t1_b2"].reshape(1, -1)),
        "shB1": pack_b1(inputs["sh_b1"]), "shB2": f32(inputs["sh_b2"].reshape(1, -1)),
        "t2B1": pack_b1(inputs["t2_b1"]), "t2B2": f32(inputs["t2_b2"].reshape(1, -1)),
        "gshW": pack_gw(inputs["gsh_W"]),
        "g1W": pack_gw(inputs["g1_W"]),
        "g2W": pack_gw(inputs["g2_W"]),
        "gB": f32(gb.reshape(96, 1)),
    }


def _pack_xT(x):
    """[BC, D] slice -> [128, (d_tile, b)] packed transpose."""
    nd = x.shape[1] // 128
    return np.ascontiguousarray(
        x.T.reshape(nd, 128, BC).transpose(1, 0, 2).reshape(128, nd * BC),
        dtype=np.float32)


def kernel(**inputs):
    global _NC_CACHE
    if _NC_CACHE is None:
        _NC_CACHE = _build_nc()
    nc = _NC_CACHE

    shared = _pack_inputs(inputs)
    xf, x1, x2 = inputs["x_full"], inputs["x_task1"], inputs["x_task2"]

    in_maps = []
    for c in range(N_CORES):
        rows = slice(c * BC, (c + 1) * BC)
        m = dict(shared)
        m["xfT"] = _pack_xT(xf[rows])
        m["x1T"] = _pack_xT(x1[rows])
        m["x2T"] = _pack_xT(x2[rows])
        in_maps.append(m)

    res = run_bass_kernel_spmd(nc, in_maps, list(range(N_CORES)))
    out_sh = np.concatenate([res.results[c]["out_sh"] for c in range(N_CORES)])
    out1 = np.concatenate([res.results[c]["out1"] for c in range(N_CORES)])
    out2 = np.concatenate([res.results[c]["out2"] for c in range(N_CORES)])
    return (out_sh, out1, out2)


# revision 33
# speedup vs baseline: 1.0001x; 1.0001x over previous
"""CGC MoE routing kernel for Trainium2, 8-core data-parallel over batch.

Problem (per reference):
  B=4096, D_FULL=1024, D_T1=D_T2=512, experts: 4 shared (on x_full),
  4 task-1 (on x_task1), 4 task-2 (on x_task2); each expert is a 2-layer
  ReLU MLP (hidden 512, out 256). Three softmax gates combine expert
  outputs into (out_sh, out1, out2), each [4096, 256] fp32.

Strategy: shard the batch 8 ways (512 rows/core), replicate weights.
Each core computes all 12 experts + gates for its shard; host concats.
Matmuls run in MM_DT (bfloat16 by default, ~3e-3 rel err; float32r gives
~2e-4 at ~20% more time) with fp32 PSUM accumulation.  Matmul operands
are cast from fp32 during their SWDGE DMA; DMAs are heavily batched
because each SWDGE issue costs ~1us on the GpSimd engine.

Layout: activations kept as [feature, batch]; the host pre-transposes
the x shards and pre-packs weights into SBUF partition layout so every
DMA is a long contiguous run per partition.
  L1: H[h,b]  = sum_d W1[d,h].T @ X'[d,b]      (W1 tiles stationary)
  L2: EO[b,o] = sum_h H[h,b].T  @ W2[h,o]      (H tiles stationary)
      + ones[1,b].T @ b2[1,o] K=1 matmul for the bias, then ReLU.
  Gates: logits[g,b] = sum_d gW[d,g].T @ X'[d,b]; exp with per-partition
      bias on ACT; PE-transpose to [b,g]; columns pre-scaled by 1/rowsum
      so the combine uses normalized gates directly.
  Combine: acc[b,o] (+)= EO_e[b,o] * gate_col[b,1] on DVE
      (scalar_tensor_tensor fused multiply-add); acc DMAs straight out.
"""
import os as _os
import sys
import numpy as np

sys.path.insert(0, "/opt/trn_rl_repo")

import concourse.bass as bass
import concourse.mybir as mybir
import concourse.tile as tile
import concourse.masks as masks
from concourse.bass_utils import run_bass_kernel_spmd

F32 = mybir.dt.float32
MM_DT = (mybir.dt.float32r if _os.environ.get("MOE_MM_DT") == "fp32r"
         else mybir.dt.bfloat16)

B = 4096
N_CORES = 8
BC = B // N_CORES          # 512 rows per core
DF, D1, D2 = 1024, 512, 512
HID, OUT = 512, 256
NB = BC // 128             # 4 batch tiles per core
NH = HID // 128            # 4 hidden tiles
E = 4                      # experts per group
NDS = {"t1": D1 // 128, "sh": DF // 128, "t2": D2 // 128}
W1SZ = {g: nd * HID for g, nd in NDS.items()}
WSZ = {g: W1SZ[g] + NH * OUT for g in NDS}   # combined w1+w2 free size


def _legalize_waits(nc, max_waits: int = 1):
    """This walrus build supports a single sync wait per instruction;
    hoist extra waits onto standalone single-wait EventSemaphore
    instructions inserted just before (same engine, same order)."""
    uid = 0
    for f in nc.m.functions:
        for blk in f.blocks:
            out = []
            changed = False
            for inst in blk.instructions:
                si = inst.sync_info
                ow = list(si.on_wait) if si and si.on_wait else []
                if len(ow) > max_waits:
                    changed = True
                    for w in ow[:-max_waits]:
                        ev = mybir.InstEventSemaphore(
                            name=f"legalw-{uid}",
                            sync_info=mybir.SyncInfo(on_wait=[w], on_update=[]),
                        )
                        uid += 1
                        ev.engine = inst.engine
                        out.append(ev)
                    inst.sync_info = mybir.SyncInfo(
                        on_wait=ow[-max_waits:],
                        on_update=list(si.on_update) if si.on_update else [],
                    )
                out.append(inst)
            if changed:
                blk.instructions = out
    return nc


def _build_nc():
    nc = bass.Bass()

    def din(name, shape):
        return nc.declare_dram_parameter(name, list(shape), F32, isOutput=False)

    # x shards packed [128, (d_tile, b)]
    xfT = din("xfT", (128, NDS["sh"] * BC))
    x1T = din("x1T", (128, NDS["t1"] * BC))
    x2T = din("x2T", (128, NDS["t2"] * BC))
    # per-expert combined weights [E, 128, (nd*HID + NH*OUT)]
    t1W = din("t1W", (E, 128, WSZ["t1"]))
    shW = din("shW", (E, 128, WSZ["sh"]))
    t2W = din("t2W", (E, 128, WSZ["t2"]))
    # biases: b1 packed per group [128, E*NH]; b2 per group [E, OUT]
    t1B1 = din("t1B1", (128, E * NH)); t1B2 = din("t1B2", (1, E * OUT))
    shB1 = din("shB1", (128, E * NH)); shB2 = din("shB2", (1, E * OUT))
    t2B1 = din("t2B1", (128, E * NH)); t2B2 = din("t2B2", (1, E * OUT))
    # gate weights packed [128, nd*ng]; all gate biases packed [28, 1]
    gshW = din("gshW", (128, NDS["sh"] * 12))
    g1W = din("g1W", (128, NDS["t1"] * 8))
    g2W = din("g2W", (128, NDS["t2"] * 8))
    gB = din("gB", (96, 1))

    out_sh = nc.declare_dram_parameter("out_sh", [BC, OUT], F32, isOutput=True)
    out1 = nc.declare_dram_parameter("out1", [BC, OUT], F32, isOutput=True)
    out2 = nc.declare_dram_parameter("out2", [BC, OUT], F32, isOutput=True)

    with tile.TileContext(nc) as tc:
        _emit(nc, tc,
              {"xf": xfT, "x1": x1T, "x2": x2T},
              # expert groups in processing order: t1, sh, t2
              [("t1", t1W, t1B1, t1B2),
               ("sh", shW, shB1, shB2),
               ("t2", t2W, t2B1, t2B2)],
              [(gshW, 12), (g1W, 8), (g2W, 8)], gB,
              [out_sh, out1, out2])
    _legalize_waits(nc)
    return nc


def _emit(nc, tc, xins, expert_groups, gate_params, gB, outs):
    from contextlib import ExitStack
    ctx = ExitStack()
    with ctx:
        xp = ctx.enter_context(tc.tile_pool(name="xp", bufs=1))
        wp = ctx.enter_context(tc.tile_pool(name="wp", bufs=3))
        bp = ctx.enter_context(tc.tile_pool(name="bp", bufs=1))
        hp = ctx.enter_context(tc.tile_pool(name="hp", bufs=2))
        eop = ctx.enter_context(tc.tile_pool(name="eop", bufs=12))
        gp = ctx.enter_context(tc.tile_pool(name="gp", bufs=1))
        accp = ctx.enter_context(tc.tile_pool(name="accp", bufs=1))
        misc = ctx.enter_context(tc.tile_pool(name="misc", bufs=1))
        # PSUM: 8 banks; L1 and gate logits share tag p1, L2 and gate
        # transposes share tag p2.
        ps1 = ctx.enter_context(tc.tile_pool(name="ps1", bufs=4, space="PSUM"))
        ps2 = ctx.enter_context(tc.tile_pool(name="ps2", bufs=4, space="PSUM"))

        # ---- batched loads ---------------------------------------------
        def load_xT(key, hwdge=False):
            n = NDS[{"x1": "t1", "xf": "sh", "x2": "t2"}[key]] * BC
            t = xp.tile([128, n], MM_DT, name=f"x_{key}", tag=f"x_{key}")
            if hwdge:
                t32 = xp.tile([128, n], F32, name=f"x32_{key}",
                              tag=f"x32_{key}")
                nc.sync.dma_start(t32[:], xins[key][:])
                half = n // 2
                nc.vector.tensor_copy(t[:, :half], t32[:, :half])
                nc.vector.tensor_copy(t[:, half:], t32[:, half:])
            else:
                nc.gpsimd.dma_start(t[:], xins[key][:])
            return t

        def load_w(W, group, e):
            w = wp.tile([128, WSZ[group]], MM_DT, name="w", tag="w")
            nc.gpsimd.dma_start(w[:], W[e])
            return w

        # x1 + first expert's weights lead the SWDGE queue; the first
        # expert's w1 is loaded per-d-slice so its (d-outer) layer 1 can
        # start after only x1 + one 256 KB slice.
        x1 = load_xT("x1")
        xts = {"t1": x1}
        g0, W_0, B1_0, B2_0 = expert_groups[0]
        w_e0 = wp.tile([128, WSZ[g0]], MM_DT, name="w", tag="w")
        for di in range(NDS[g0]):
            nc.gpsimd.dma_start(
                w_e0[:, di * HID:(di + 1) * HID],
                W_0[0][:, di * HID:(di + 1) * HID])
        nc.gpsimd.dma_start(w_e0[:, W1SZ[g0]:], W_0[0][:, W1SZ[g0]:])
        xts["sh"] = load_xT("xf", hwdge=True)
        xts["t2"] = load_xT("x2", hwdge=True)

        # biases (HWDGE, fp32) and b2 (SWDGE, MM_DT) per group, one DMA each
        b1t, b2t = {}, {}
        for group, W, B1, B2 in expert_groups:
            b1t[group] = bp.tile([128, E * NH], F32, name=f"b1_{group}",
                                 tag=f"b1_{group}")
            nc.sync.dma_start(b1t[group][:], B1[:])
            b2t[group] = bp.tile([1, E * OUT], MM_DT, name=f"b2_{group}",
                                 tag=f"b2_{group}")
            nc.gpsimd.dma_start(b2t[group][:], B2[:])
        gbt = bp.tile([96, 1], F32, name="gbt", tag="gbt")
        nc.sync.dma_start(gbt[:], gB[:])
        gb_sl = {0: gbt[0:12, :], 1: gbt[32:40, :], 2: gbt[64:72, :]}

        # identity for PE transpose; ones row for K=1 bias matmuls
        # (memset/iota are invalid ISA at fp32r/bf16 -> build fp32 + copy)
        ident32 = misc.tile([128, 128], F32, tag="ident32")
        masks.make_identity(nc, ident32[:])
        ident = misc.tile([128, 128], MM_DT, tag="ident")
        nc.vector.tensor_copy(ident[:], ident32[:])
        ones32 = misc.tile([1, 128], F32, tag="ones32")
        nc.vector.memset(ones32[:], 1.0)
        ones = misc.tile([1, 128], MM_DT, tag="ones")
        nc.vector.tensor_copy(ones[:], ones32[:])

        # accumulator tiles [128, OUT] per output per b-tile
        acc = [[accp.tile([128, OUT], F32, name=f"acc{o}_{bi}",
                          tag=f"acc{o}_{bi}")
                for bi in range(NB)] for o in range(3)]
        acc_init = [[False] * NB for _ in range(3)]

        # expert -> (output index, gate set, gate column) contributions
        # gates: gsh over [t1(0-3), t2(4-7), sh(8-11)]
        #        g1  over [t1(0-3), sh(4-7)]; g2 over [t2(0-3), sh(4-7)]
        def contributions(group, e):
            if group == "t1":
                return [(0, 0, e), (1, 1, e)]
            elif group == "t2":
                return [(0, 0, 4 + e), (2, 2, e)]
            else:
                return [(0, 0, 8 + e), (1, 1, 4 + e), (2, 2, 4 + e)]

        # processing order: t1(0..3), sh(0..3), t2(0..3)
        def _is_last_contrib(group, e, o):
            if o == 1:
                return group == "sh" and e == E - 1
            return group == "t2" and e == E - 1

        # ---- expert bodies ---------------------------------------------
        def emit_expert_l1(group, e, w):
            xt = xts[group]
            nd = NDS[group]
            b1 = b1t[group]
            h = hp.tile([128, NH * BC], MM_DT, name="h", tag="h")
            for hi in range(NH):
                p1 = ps1.tile([128, BC], F32, name="p1", tag="p1")
                for di in range(nd):
                    nc.tensor.matmul(
                        p1[:], w[:, di * HID + hi * 128: di * HID + (hi + 1) * 128],
                        xt[:, di * BC:(di + 1) * BC],
                        start=(di == 0), stop=(di == nd - 1))
                nc.scalar.activation(h[:, hi * BC:(hi + 1) * BC], p1[:],
                                     mybir.ActivationFunctionType.Relu,
                                     bias=b1[:, e * NH + hi: e * NH + hi + 1])
            return h

        def emit_expert_l2c(group, e, h, w):
            w2off = W1SZ[group]
            eos = []
            for bi in range(NB):
                p2 = ps2.tile([128, OUT], F32, name="p2", tag="p2")
                for hi in range(NH):
                    nc.tensor.matmul(
                        p2[:],
                        h[:, hi * BC + bi * 128: hi * BC + (bi + 1) * 128],
                        w[:, w2off + hi * OUT: w2off + (hi + 1) * OUT],
                        start=(hi == 0), stop=False)
                nc.tensor.matmul(p2[:], ones[:],
                                 b2t[group][:, e * OUT:(e + 1) * OUT],
                                 start=False, stop=True)
                eo = eop.tile([128, OUT], F32, name="eo", tag="eo")
                nc.scalar.activation(eo[:], p2[:],
                                     mybir.ActivationFunctionType.Relu)
                eos.append(eo)
            return eos

        def emit_combine(group, e, eos, gate_cols):
            for bi in range(NB):
                eo = eos[bi]
                for (o, gs, col) in contributions(group, e):
                    g_ap = gate_cols[gs][bi][:, col:col + 1]
                    a = acc[o][bi]
                    if not acc_init[o][bi]:
                        nc.vector.tensor_scalar_mul(a[:], eo[:], g_ap)
                        acc_init[o][bi] = True
                    else:
                        nc.vector.scalar_tensor_tensor(
                            a[:], eo[:], g_ap, a[:],
                            op0=mybir.AluOpType.mult,
                            op1=mybir.AluOpType.add)
                    if _is_last_contrib(group, e, o):
                        nc.sync.dma_start(
                            outs[o][bi * 128:(bi + 1) * 128, :], a[:])

        # ---- gates ------------------------------------------------------
        def emit_gates():
            gate_cols = []
            for gi, (gW, ng) in enumerate(gate_params):
                xt = {0: xts["sh"], 1: xts["t1"], 2: xts["t2"]}[gi]
                nd = {0: NDS["sh"], 1: NDS["t1"], 2: NDS["t2"]}[gi]
                gwt = gp.tile([128, nd * ng], MM_DT, name=f"gw{gi}",
                              tag=f"gw{gi}")
                nc.gpsimd.dma_start(gwt[:], gW[:])
                lg = ps1.tile([ng, BC], F32, name="lg", tag="p1")
                for di in range(nd):
                    nc.tensor.matmul(
                        lg[:], gwt[:, di * ng:(di + 1) * ng],
                        xt[:, di * BC:(di + 1) * BC],
                        start=(di == 0), stop=(di == nd - 1))
                eg = gp.tile([ng, BC], MM_DT, name=f"eg{gi}", tag=f"eg{gi}")
                nc.scalar.activation(eg[:], lg[:],
                                     mybir.ActivationFunctionType.Exp,
                                     bias=gb_sl[gi])
                cols = []
                for bi in range(NB):
                    pt = ps2.tile([128, ng], MM_DT, name="gtr", tag="p2")
                    nc.tensor.transpose(pt[:], eg[:, bi * 128:(bi + 1) * 128],
                                        ident[:ng, :ng])
                    ct = gp.tile([128, ng], F32, name=f"gc{gi}_{bi}",
                                 tag=f"gc{gi}_{bi}")
                    nc.vector.tensor_copy(ct[:], pt[:])
                    st = gp.tile([128, 1], F32, name=f"gs{gi}_{bi}",
                                 tag=f"gs{gi}_{bi}")
                    nc.vector.tensor_reduce(st[:], ct[:],
                                            axis=mybir.AxisListType.X,
                                            op=mybir.AluOpType.add)
                    rt = gp.tile([128, 1], F32, name=f"gr{gi}_{bi}",
                                 tag=f"gr{gi}_{bi}")
                    nc.vector.reciprocal(rt[:], st[:])
                    # pre-scale: combine then uses normalized gates directly
                    nc.vector.tensor_scalar_mul(ct[:], ct[:], rt[:])
                    cols.append(ct)
                gate_cols.append(cols)
            return gate_cols

        # ---- emission order --------------------------------------------
        # t1_0 L1/L2 first (PE starts as soon as x1+w arrive), then t1_1,
        # then gates (xf has landed by then), deferred combines, rest.
        h_0 = emit_expert_l1(g0, 0, w_e0)
        eos_0 = emit_expert_l2c(g0, 0, h_0, w_e0)
        w_e1 = load_w(W_0, g0, 1)
        h_1 = emit_expert_l1(g0, 1, w_e1)
        eos_1 = emit_expert_l2c(g0, 1, h_1, w_e1)
        gate_cols = emit_gates()
        emit_combine(g0, 0, eos_0, gate_cols)
        emit_combine(g0, 1, eos_1, gate_cols)
        for gidx, (group, W, B1, B2) in enumerate(expert_groups):
            for e in range(E):
                if gidx == 0 and e in (0, 1):
                    continue
                w = load_w(W, group, e)
                h = emit_expert_l1(group, e, w)
                eos = emit_expert_l2c(group, e, h, w)
                emit_combine(group, e, eos, gate_cols)


_NC_CACHE = None


def _pack_inputs(inputs):
    """Host-side packing into SBUF partition layouts (pure relayout)."""
    f32 = lambda a: np.ascontiguousarray(a, dtype=np.float32)

    def pack_w(w1, w2):   # [E,D,HID],[E,HID,OUT] -> [E,128,nd*HID+NH*OUT]
        e, dd, _ = w1.shape
        nd = dd // 128
        a = w1.reshape(e, nd, 128, HID).transpose(0, 2, 1, 3).reshape(e, 128, nd * HID)
        b = w2.reshape(e, NH, 128, OUT).transpose(0, 2, 1, 3).reshape(e, 128, NH * OUT)
        return f32(np.concatenate([a, b], axis=2))

    def pack_b1(b):       # [E, HID] -> [128, E*NH] with [p, e*NH+n] = b[e, n*128+p]
        e, hh = b.shape
        nh = hh // 128
        return f32(b.reshape(e, nh, 128).transpose(2, 0, 1).reshape(128, e * nh))

    def pack_gw(w):       # [D, ng] -> [128, nd*ng]
        dd, ng = w.shape
        nd = dd // 128
        return f32(w.reshape(nd, 128, ng).transpose(1, 0, 2).reshape(128, nd * ng))

    gb = np.zeros(96, dtype=np.float32)
    gb[0:12] = inputs["gsh_b"]; gb[32:40] = inputs["g1_b"]; gb[64:72] = inputs["g2_b"]
    return {
        "t1W": pack_w(inputs["t1_W1"], inputs["t1_W2"]),
        "shW": pack_w(inputs["sh_W1"], inputs["sh_W2"]),
        "t2W": pack_w(inputs["t2_W1"], inputs["t2_W2"]),
        "t1B1": pack_b1(inputs["t1_b1"]), "t1B2": f32(inputs["t1_b2"].reshape(1, -1)),
        "shB1": pack_b1(inputs["sh_b1"]), "shB2": f32(inputs["sh_b2"].reshape(1, -1)),
        "t2B1": pack_b1(inputs["t2_b1"]), "t2B2": f32(inputs["t2_b2"].reshape(1, -1)),
        "gshW": pack_gw(inputs["gsh_W"]),
        "g1W": pack_gw(inputs["g1_W"]),
        "g2W": pack_gw(inputs["g2_W"]),
        "gB": f32(gb.reshape(96, 1)),
    }


def _pack_xT(x):
    """[BC, D] slice -> [128, (d_tile, b)] packed transpose."""
    nd = x.shape[1] // 128
    return np.ascontiguousarray(
        x.T.reshape(nd, 128, BC).transpose(1, 0, 2).reshape(128, nd * BC),
        dtype=np.float32)


def kernel(**inputs):
    global _NC_CACHE
    if _NC_CACHE is None:
        _NC_CACHE = _build_nc()
    nc = _NC_CACHE

    shared = _pack_inputs(inputs)
    xf, x1, x2 = inputs["x_full"], inputs["x_task1"], inputs["x_task2"]

    in_maps = []
    for c in range(N_CORES):
        rows = slice(c * BC, (c + 1) * BC)
        m = dict(shared)
        m["xfT"] = _pack_xT(xf[rows])
        m["x1T"] = _pack_xT(x1[rows])
        m["x2T"] = _pack_xT(x2[rows])
        in_maps.append(m)

    res = run_bass_kernel_spmd(nc, in_maps, list(range(N_CORES)))
    out_sh = np.concatenate([res.results[c]["out_sh"] for c in range(N_CORES)])
    out1 = np.concatenate([res.results[c]["out1"] for c in range(N_CORES)])
    out2 = np.concatenate([res.results[c]["out2"] for c in range(N_CORES)])
    return (out_sh, out1, out2)
